# revision 2
# baseline (speedup 1.0000x reference)
"""Trainium2 Bass kernel for a fake-quantized MLP (qlinear -> gelu -> qlinear).

v3. Same exact-int8 emulation semantics as the baseline, but:
  - every bulk DMA stream alternates between the two HWDGE queues
    (qSP / qAct) -- one queue tops out ~208 GB/s, two reach ~350.
  - front-end restructured: x scan (sync queue) overlaps w1 scan; the x
    AllReduce rides under the w1 scan; chunks 0-1 of x are transposed
    during the scan (stashed fp16) so phase A starts right after the w1
    quantize (~145us vs ~262us).
  - all psum->sbuf copies after PE transposes are batched strided DVE
    copies (4 blocks per op) instead of per-block ACT copies.
  - h staged to DRAM in fp16 (halves h traffic; validated: no extra err).
  - w2 scan runs early in phase A, w2 quant+transpose trickles through
    chunks 2-3, so the phase A/B boundary only pays AllReduce(hmax) +
    h-quantize (~35us vs ~133us).
  - h chunk-0 tiles prefetched before the AllReduce.
  - DMA queues split by stream: scans+stores on sync, x/w1 loads on
    scalar, w2/h loads on vector, collective bounces on gpsimd.
"""

import sys

import numpy as np

try:
    import concourse.bass as bass
except ImportError:  # pragma: no cover
    sys.path.insert(0, "/opt/trn_rl_repo")
    import concourse.bass as bass

import concourse.mybir as mybir
from contextlib import ExitStack
import concourse.tile as tile
from concourse import masks
from concourse.bass_utils import run_bass_kernel_spmd

from concourse.bass import _add_dep_helper as _add_dep

F32 = mybir.dt.float32
F16 = mybir.dt.float16
BF16 = mybir.dt.bfloat16
AF = mybir.ActivationFunctionType
ALU = mybir.AluOpType

QP = 127.0
EPS = 1e-6
MAGIC = 12582912.0  # 1.5 * 2**23: fp32 round-to-nearest-even integer trick

B, S, C, H = 32, 1024, 1024, 4096
N_CORES = 8


def _split_matmul_waits(nc):
    """This toolchain's walrus codegen allows only ONE sync-wait slot per
    lowered instruction (Matmult waits all land on its LDWEIGHTS since
    --enable-ldw-opt=false; queue DMAs use a single-slot DIRECT2D struct).
    Peel extra waits onto same-engine NoOps inserted just before, except for
    framework-generated drain/barrier instructions which support many."""
    n_split = 0
    for f in nc.m.functions:
        for bb in f.blocks:
            insts = bb.instructions
            out = []
            changed = False
            for inst in insts:
                si = getattr(inst, "sync_info", None)
                if si is not None and si.on_wait and len(si.on_wait) > 1:
                    waits = list(si.on_wait)
                    for k, w in enumerate(waits[:-1]):
                        nop = mybir.InstNoOp(
                            name=f"{inst.name}-wsplit{k}", ins=[], outs=[]
                        )
                        nop.engine = inst.engine
                        nop.sync_info = mybir.SyncInfo(
                            on_wait=[w], on_update=[]
                        )
                        out.append(nop)
                    inst.sync_info = mybir.SyncInfo(
                        on_wait=[waits[-1]], on_update=list(si.on_update or [])
                    )
                    n_split += 1
                    changed = True
                out.append(inst)
            if changed:
                bb.instructions = out
    return n_split


def _dedup_ldweights(nc):
    """Tile legalization emits explicit Ldweights+Matmult pairs, and walrus
    runs with --enable-ldw-opt=false, so every matmul re-streams its
    stationary operand.  Drop an Ldweights whose weights AP is identical to
    the previous one on the PE stream; keep its semaphore effects on a
    NoOp."""
    n = 0
    for f in nc.m.functions:
        for bb in f.blocks:
            insts = bb.instructions
            out = []
            last_key = None
            changed = False
            for inst in insts:
                if isinstance(inst, mybir.InstLdweights):
                    key = str(inst.ins[0])
                    if key == last_key:
                        si = getattr(inst, "sync_info", None)
                        if si is not None and (si.on_wait or si.on_update):
                            nop = mybir.InstNoOp(
                                name=inst.name + "-lw", ins=[], outs=[]
                            )
                            nop.engine = inst.engine
                            nop.sync_info = si
                            out.append(nop)
                        n += 1
                        changed = True
                        continue
                    last_key = key
                elif isinstance(inst, mybir.InstMatmult):
                    if inst.is_transpose or getattr(inst, "ldweights", None):
                        last_key = None
                out.append(inst)
            if changed:
                bb.instructions = out
    return n


def build_nc(rows=4096, c=C, h=H, n_cores=N_CORES, gelu="Gelu", split_waits=True):
    assert rows % 1024 == 0 and c % 512 == 0 and h % 512 == 0
    nc = bass.Bass()

    x_in = nc.dram_tensor("x", [rows, c], F32, kind="ExternalInput")
    w1_in = nc.dram_tensor("w1", [h, c], F32, kind="ExternalInput")
    b1_in = nc.dram_tensor("b1", [h], F32, kind="ExternalInput")
    w2_in = nc.dram_tensor("w2", [c, h], F32, kind="ExternalInput")
    b2_in = nc.dram_tensor("b2", [c], F32, kind="ExternalInput")
    y_out = nc.dram_tensor("y", [rows, c], F32, kind="ExternalOutput")

    ct = c // 128    # 8
    ht = h // 128    # 32
    CH = 1024        # phase A m-chunk
    n_ms = CH // 512  # 2
    n_cha = rows // CH   # 4 phase A chunks
    PB = 512         # phase B m-chunk
    n_chb = rows // PB   # 8 phase B chunks
    groups = [list(range(n_cores))]

    with tile.TileContext(nc) as tc, ExitStack() as top:
        consts = top.enter_context(tc.tile_pool(name="consts", bufs=1))
        scal = top.enter_context(tc.tile_pool(name="scal", bufs=1))
        dram = top.enter_context(tc.tile_pool(name="dram", bufs=1, space="DRAM"))

        ident = consts.tile([128, 128], BF16)
        masks.make_identity(nc, ident[:])
        ident_f = consts.tile([128, 128], F32)
        masks.make_identity(nc, ident_f[:])

        b1_sb = consts.tile([128, ht], F32)
        nc.sync.dma_start(
            out=b1_sb[:], in_=b1_in.ap().rearrange("(a b) -> b a", b=128)
        )
        magic_b = consts.tile([128, 1], F32)
        nc.vector.memset(magic_b[:], MAGIC)

        h_dram = dram.tile([h, rows], F16)
        arx_in = dram.tile([1, 1], F32, tag="arxi")
        arx_out = dram.tile([1, 1], F32, tag="arxo")
        arh_in = dram.tile([1, 1], F32, tag="arhi")
        arh_out = dram.tile([1, 1], F32, tag="arho")
        arw_b = dram.tile([2, 1], F32, tag="arwb")  # sw1/sw2 bcast bounce

        # persistent quantized operand tiles
        mid = ExitStack()  # layer-1 operands: closed after phase A
        w1qT_pool = mid.enter_context(tc.tile_pool(name="w1qT", bufs=1, side="right"))
        w1qT = w1qT_pool.tile([128, ct * h], BF16)  # (cb, h) 64KB/part
        xqtp = mid.enter_context(tc.tile_pool(name="xqT", bufs=2, side="right"))
        xqT = {}

        def _derive(bcast_src_dram, name):
            # broadcast via the gpsimd SWDGE queue: a collective-gated DMA on
            # an HWDGE FIFO would block every bulk transfer queued behind it
            b = scal.tile([128, 1], F32, name=name + "_b")
            nc.gpsimd.dma_start(out=b[:], in_=bcast_src_dram.to_broadcast((128, 1)))
            s = scal.tile([128, 1], F32, name="s_" + name)
            nc.vector.tensor_scalar(
                out=s[:], in0=b[:], scalar1=EPS, scalar2=float(1.0 / QP),
                op0=ALU.max, op1=ALU.mult,
            )
            inv = scal.tile([128, 1], F32, name="inv_" + name)
            nc.vector.reciprocal(out=inv[:], in_=s[:])
            return s, inv

        def _preduce(acc, pspool, tag, bufs=1):
            pt = pspool.tile([1, 128], F32, tag=tag, bufs=bufs)
            nc.tensor.matmul(
                pt[:], lhsT=acc[:], rhs=ident_f[:], start=True, stop=True
            )
            out = scal.tile([1, 1], F32, name=tag + "_r")
            nc.vector.tensor_reduce(
                out=out[:], in_=pt[:], axis=mybir.AxisListType.X, op=ALU.max
            )
            return out

        xmax = scal.tile([128, 1], F32)
        wmax1 = scal.tile([128, 1], F32)
        wmax2 = scal.tile([128, 1], F32)
        hmax = scal.tile([128, 1], F32)
        nc.vector.memset(xmax[:], 0.0)
        nc.vector.memset(wmax1[:], 0.0)
        nc.vector.memset(wmax2[:], 0.0)
        nc.vector.memset(hmax[:], 0.0)

        # ---------------- front: scans + stash + quantize ----------------
        front = ExitStack()
        xT0p = front.enter_context(tc.tile_pool(name="xT0", bufs=1))
        xT0 = xT0p.tile([128, ct * CH], F32)  # chunk-0 stash, fp32 (32KB/part)

        W1_KEEP = 6   # [128,2048] w1 scan tiles kept resident (rb 20..31)
        w0k = front.enter_context(tc.tile_pool(name="w0k", bufs=W1_KEEP))
        with tc.tile_pool(name="x0", bufs=3) as x0p, tc.tile_pool(
            name="w0", bufs=2
        ) as w0p, tc.tile_pool(name="sc0", bufs=4) as sc0, tc.tile_pool(
            name="pF", bufs=2, space="PSUM"
        ) as pF:
            # x scan: 32 plain [128, 1024] row-block tiles
            for mb in range(rows // 128):
                t = x0p.tile([128, c], F32, tag="x0a")
                eng = nc.sync if mb % 2 == 0 else nc.scalar
                eng.dma_start(out=t[:], in_=x_in[mb * 128 : (mb + 1) * 128, :])
                r = sc0.tile([128, 1], F32, tag="x0r")
                nc.vector.tensor_reduce(
                    out=r[:], in_=t[:], axis=mybir.AxisListType.X, op=ALU.max,
                    apply_absolute_value=True,
                )
                nc.vector.tensor_tensor(out=xmax[:], in0=xmax[:], in1=r[:],
                                        op=ALU.max)
                if mb < CH // 128:  # stash chunk 0 transposed (fp32, exact)
                    for cq in range(ct // 4):
                        ps = pF.tile([128, 512], F32, tag="pT", bufs=2)
                        for i in range(4):
                            cb = cq * 4 + i
                            nc.tensor.matmul(
                                ps[:, i * 128 : (i + 1) * 128],
                                lhsT=t[:, cb * 128 : (cb + 1) * 128],
                                rhs=ident_f[:], start=True, stop=True,
                            )
                        nc.vector.tensor_copy(
                            out=xT0[:].rearrange(
                                "p (cb m) -> p cb m", cb=ct
                            )[:, cq * 4 : (cq + 1) * 4,
                              mb * 128 : (mb + 1) * 128],
                            in_=ps[:].rearrange("p (a b) -> p a b", a=4),
                        )
            with tc.high_priority():
                xmax_r = _preduce(xmax, pF, "pR")  # bufs=1 tag
                nc.gpsimd.dma_start(out=arx_in[:], in_=xmax_r[:])
                nc.gpsimd.collective_compute(
                    "AllReduce", ALU.max, replica_groups=groups,
                    ins=[arx_in.opt()], outs=[arx_out.opt()],
                )

            # w1 scan: 16 tiles of [128, 2*1024]; the last N_KEEP stay
            # resident so only rb < 2*(16-N_KEEP) needs a reload pass.
            w1_keep = {}
            for rb2 in range(h // 256):
                keep = rb2 >= 16 - W1_KEEP
                if keep:
                    t = w0k.tile([128, 2 * c], F32, tag="w0k",
                                 name=f"w0k{rb2}")
                    w1_keep[rb2] = t
                else:
                    t = w0p.tile([128, 2 * c], F32, tag="w0a")
                eng = nc.sync if rb2 % 2 == 0 else nc.scalar
                eng.dma_start(
                    out=t[:].rearrange("b (a c) -> b a c", a=2),
                    in_=w1_in[rb2 * 256 : (rb2 + 1) * 256, :].rearrange(
                        "(a b) c -> b a c", b=128
                    ),
                )
                r = sc0.tile([128, 1], F32, tag="w0r")
                nc.vector.tensor_reduce(
                    out=r[:], in_=t[:], axis=mybir.AxisListType.X, op=ALU.max,
                    apply_absolute_value=True,
                )
                nc.vector.tensor_tensor(out=wmax1[:], in0=wmax1[:], in1=r[:],
                                        op=ALU.max)
            w1max_r = _preduce(wmax1, pF, "pR")
            nc.gpsimd.dma_start(out=arw_b[0:1, :], in_=w1max_r[:])
            sw1, inv_sw1 = _derive(arw_b[0:1, :], "w1")
            sx, inv_sx = _derive(arx_out, "x")
            sxw1 = scal.tile([128, 1], F32)
            nc.vector.tensor_tensor(out=sxw1[:], in0=sx[:], in1=sw1[:],
                                    op=ALU.mult)

        # scan pools closed.  xT0 quant runs entirely on DVE (psum-staged)
        # so the AllReduce gate never blocks the ACT queue; w1 quant does the
        # kept scan tiles first (no DMA), then the rb<20 reload as it lands.
        with tc.tile_pool(name="w1rl", bufs=2) as w1rl, tc.tile_pool(
            name="w1q", bufs=2
        ) as w1qp, tc.tile_pool(name="pG", bufs=2, space="PSUM") as pG:
            xqT[0] = xqtp.tile([128, ct * CH], BF16, tag="xqT", name="xqT0")

            def _w1_quant(rb, wt_ap):
                nc.scalar.activation(
                    out=wt_ap, in_=wt_ap, func=AF.Identity,
                    bias=magic_b[:], scale=inv_sw1[:],
                )
                wq = w1qp.tile([128, c], BF16, tag="w1b")
                nc.vector.tensor_scalar_add(out=wq[:], in0=wt_ap,
                                            scalar1=-MAGIC)
                for cq in range(ct // 4):
                    ps = pG.tile([128, 512], F32, tag="pT", bufs=2)
                    for i in range(4):
                        cb = cq * 4 + i
                        nc.tensor.matmul(
                            ps[:, i * 128 : (i + 1) * 128],
                            lhsT=wq[:, cb * 128 : (cb + 1) * 128],
                            rhs=ident[:], start=True, stop=True,
                        )
                    nc.vector.tensor_copy(
                        out=w1qT[:].rearrange("p (cb h) -> p cb h", cb=ct)[
                            :, cq * 4 : (cq + 1) * 4,
                            rb * 128 : (rb + 1) * 128],
                        in_=ps[:].rearrange("p (a b) -> p a b", a=4),
                    )

            # Reload dma issues go on the sync queue ONLY: the ACT engine
            # must never gate a reload issue (its quant ops wait on data).
            # Kept-tile quants run first (their data arrived with the scan),
            # then reload quants trail the transfers.
            n_reload = 2 * (16 - W1_KEEP)
            rl_tiles = {}
            for rb in range(min(4, n_reload)):
                wt = w1rl.tile([128, c], F32, tag="w1f", name=f"w1f{rb}")
                nc.sync.dma_start(
                    out=wt[:], in_=w1_in[rb * 128 : (rb + 1) * 128, :]
                )
                rl_tiles[rb] = wt
            # xT0 quant: DVE pass1 (scale+MAGIC -> psum), DVE pass2 (-MAGIC)
            for st in range(ct * CH // 512):
                psq = pG.tile([128, 512], F32, tag="pQ", bufs=2)
                nc.vector.tensor_scalar(
                    out=psq[:],
                    in0=xT0[:, st * 512 : (st + 1) * 512],
                    scalar1=inv_sx[:], scalar2=MAGIC,
                    op0=ALU.mult, op1=ALU.add,
                )
                nc.vector.tensor_scalar_add(
                    out=xqT[0][:, st * 512 : (st + 1) * 512],
                    in0=psq[:], scalar1=-MAGIC,
                )
            # kept w1 tiles: quantize from SBUF (no DMA)
            for rb2, t in sorted(w1_keep.items()):
                for a in range(2):
                    _w1_quant(rb2 * 2 + a, t[:, a * c : (a + 1) * c])
            for rb in range(4, n_reload):
                wt = w1rl.tile([128, c], F32, tag="w1f", name=f"w1f{rb}")
                nc.sync.dma_start(
                    out=wt[:], in_=w1_in[rb * 128 : (rb + 1) * 128, :]
                )
                rl_tiles[rb] = wt
                _w1_quant(rb - 4, rl_tiles.pop(rb - 4)[:])
            for rb in sorted(rl_tiles):
                _w1_quant(rb, rl_tiles[rb][:])
        front.close()  # xT0 + kept w1 released

        # ---------------- phase A ----------------
        # w2qT allocated only now: its 64KB/part must not overlap the
        # front's scan pools.  Lives through phase B.
        late = ExitStack()
        w2qT_pool = late.enter_context(tc.tile_pool(name="w2qT", bufs=1))
        w2qT = w2qT_pool.tile([128, ht * c], BF16)  # (jb, c) 64KB/part
        pA = ExitStack()
        xa = pA.enter_context(tc.tile_pool(name="xa", bufs=4))
        xqp = pA.enter_context(tc.tile_pool(name="xq", bufs=2))
        gs = pA.enter_context(tc.tile_pool(name="gs", bufs=4))
        gr = pA.enter_context(tc.tile_pool(name="gr", bufs=6))
        w2ld = pA.enter_context(tc.tile_pool(name="w2ld", bufs=3))
        w2qp = pA.enter_context(tc.tile_pool(name="w2q", bufs=2))
        psH = pA.enter_context(tc.tile_pool(name="psH", bufs=6, space="PSUM"))
        psT = pA.enter_context(tc.tile_pool(name="psT", bufs=2, space="PSUM"))

        def emit_xprep(ch):
            """quantize+transpose x chunk ch (>=2) from DRAM into xqT[ch].
            Loads batched 4-at-a-time so ACT-queue issue never stalls the
            next load behind a data-waiting activation."""
            xqT[ch] = xqtp.tile([128, ct * CH], BF16, tag="xqT",
                                name=f"xqT{ch}")
            for g4 in range(CH // 512):
                xts = []
                for k in range(4):
                    t8 = g4 * 4 + k
                    m0 = ch * CH + t8 * 128
                    xt = xa.tile([128, c], F32, tag="xa")
                    nc.scalar.dma_start(out=xt[:], in_=x_in[m0 : m0 + 128, :])
                    xts.append(xt)
                for k in range(4):
                    t8 = g4 * 4 + k
                    xt = xts[k]
                    nc.scalar.activation(
                        out=xt[:], in_=xt[:], func=AF.Identity,
                        bias=magic_b[:], scale=inv_sx[:],
                    )
                    xq = xqp.tile([128, c], BF16, tag="xq")
                    nc.vector.tensor_scalar_add(out=xq[:], in0=xt[:],
                                                scalar1=-MAGIC)
                    for cq in range(ct // 4):
                        ps = psT.tile([128, 512], F32, tag="psT")
                        for i in range(4):
                            cb = cq * 4 + i
                            nc.tensor.matmul(
                                ps[:, i * 128 : (i + 1) * 128],
                                lhsT=xq[:, cb * 128 : (cb + 1) * 128],
                                rhs=ident[:], start=True, stop=True,
                            )
                        nc.vector.tensor_copy(
                            out=xqT[ch][:].rearrange(
                                "p (cb m) -> p cb m", cb=ct
                            )[:, cq * 4 : (cq + 1) * 4,
                              t8 * 128 : (t8 + 1) * 128],
                            in_=ps[:].rearrange("p (a b) -> p a b", a=4),
                        )

        def emit_w2scan(batch):
            """scan 8 quarter-row-blocks of w2 for amax (batch in 0..3)."""
            for k in range(8):
                idx = batch * 8 + k
                rb, hh4 = idx // 4, idx % 4
                wt = w2ld.tile([128, c], F32, tag="w2f")
                eng = nc.sync if k % 2 == 0 else nc.scalar
                eng.dma_start(
                    out=wt[:],
                    in_=w2_in[rb * 128 : (rb + 1) * 128,
                              hh4 * 1024 : (hh4 + 1) * 1024],
                )
                r = gr.tile([128, 1], F32, tag="w2r")
                nc.vector.tensor_reduce(
                    out=r[:], in_=wt[:], axis=mybir.AxisListType.X, op=ALU.max,
                    apply_absolute_value=True,
                )
                nc.vector.tensor_tensor(out=wmax2[:], in0=wmax2[:], in1=r[:],
                                        op=ALU.max)

        def emit_w2quant(batch):
            """reload+quantize+transpose 8 quarter-blocks of w2 (batch 0..3)."""
            for k in range(8):
                q32 = batch * 8 + k
                rb, hh4 = q32 // 4, q32 % 4
                wt = w2ld.tile([128, c], F32, tag="w2f")
                eng = nc.sync if k % 2 == 0 else nc.scalar
                eng.dma_start(
                    out=wt[:],
                    in_=w2_in[rb * 128 : (rb + 1) * 128,
                              hh4 * 1024 : (hh4 + 1) * 1024],
                )
                nc.scalar.activation(
                    out=wt[:], in_=wt[:], func=AF.Identity, bias=magic_b[:],
                    scale=inv_sw2[:],
                )
                wq = w2qp.tile([128, c], BF16, tag="w2b")
                nc.vector.tensor_scalar_add(out=wq[:], in0=wt[:],
                                            scalar1=-MAGIC)
                for cq in range(2):
                    hb0 = hh4 * 8 + cq * 4
                    ps = psT.tile([128, 512], F32, tag="psT")
                    for i in range(4):
                        nc.tensor.matmul(
                            ps[:, i * 128 : (i + 1) * 128],
                            lhsT=wq[:, (cq * 4 + i) * 128 : (cq * 4 + i + 1) * 128],
                            rhs=ident[:], start=True, stop=True,
                        )
                    nc.vector.tensor_copy(
                        out=w2qT[:].rearrange("p (jb c) -> p jb c", jb=ht)[
                            :, hb0 : hb0 + 4, rb * 128 : (rb + 1) * 128],
                        in_=ps[:].rearrange("p (a b) -> p a b", a=4),
                    )

        sw2 = inv_sw2 = None
        for ch in range(n_cha):
            jb_order = list(range(ht))
            if ch == 0:
                # kept w1 rows (20..31) quantize first -- consume them first
                jb_order = list(range(2 * (16 - W1_KEEP), ht)) + list(
                    range(2 * (16 - W1_KEEP))
                )
            W2SCAN = {(0, 24): 0, (1, 8): 1, (1, 24): 2, (2, 8): 3}
            W2QUANT = {(2, 20): 0, (2, 28): 1, (3, 4): 2, (3, 12): 3}
            for jj, jb in enumerate(jb_order):
                if jj == 8 and ch <= 2:
                    emit_xprep(ch + 1)
                if (ch, jj) in W2SCAN:
                    emit_w2scan(W2SCAN[(ch, jj)])
                if ch == 2 and jj == 14:
                    w2max_r = _preduce(wmax2, psT, "psT", bufs=2)
                    nc.gpsimd.dma_start(out=arw_b[1:2, :], in_=w2max_r[:])
                    sw2, inv_sw2 = _derive(arw_b[1:2, :], "w2")
                if (ch, jj) in W2QUANT:
                    emit_w2quant(W2QUANT[(ch, jj)])
                phs = [
                    psH.tile([128, 512], F32, tag="psH",
                             name=f"psH{ch}_{jb}_{i}")
                    for i in range(n_ms)
                ]
                prev = None
                for cb in range(ct):
                    for ms in range(n_ms):
                        mmi = nc.tensor.matmul(
                            phs[ms][:],
                            lhsT=w1qT[:, cb * h + jb * 128 : cb * h + (jb + 1) * 128],
                            rhs=xqT[ch][:, cb * CH + ms * 512 : cb * CH + (ms + 1) * 512],
                            start=(cb == 0),
                            stop=(cb == ct - 1),
                        )
                        if prev is not None:
                            _add_dep(mmi.ins, prev.ins, sync=False,
                                     reason="ldw-order")
                        prev = mmi
                for ms in range(n_ms):
                    g = gs.tile([128, 512], F16, tag="gs")
                    nc.scalar.activation(
                        out=g[:], in_=phs[ms][:], func=getattr(AF, gelu),
                        bias=b1_sb[:, jb : jb + 1], scale=sxw1[:],
                    )
                    r = gr.tile([128, 1], F32, tag="gr")
                    nc.vector.tensor_reduce(
                        out=r[:], in_=g[:], axis=mybir.AxisListType.X,
                        op=ALU.max, apply_absolute_value=True,
                    )
                    nc.vector.tensor_tensor(out=hmax[:], in0=hmax[:],
                                            in1=r[:], op=ALU.max)
                    m0 = ch * CH + ms * 512
                    heng = nc.sync if jb % 2 == 0 else nc.scalar
                    heng.dma_start(
                        out=h_dram[jb * 128 : (jb + 1) * 128, m0 : m0 + 512],
                        in_=g[:],
                    )
        hmax_r = _preduce(hmax, psT, "psT", bufs=2)
        pA.close()
        mid.close()

        # ---------------- boundary: AR(hmax) + h prefetch ----------------
        pB = ExitStack()
        hb = pB.enter_context(tc.tile_pool(name="hb", bufs=10))
        hqtp = pB.enter_context(tc.tile_pool(name="hqT", bufs=2))
        ys = pB.enter_context(tc.tile_pool(name="ys", bufs=4))
        b2p = pB.enter_context(tc.tile_pool(name="b2p", bufs=1))
        psY = pB.enter_context(tc.tile_pool(name="psY", bufs=6, space="PSUM"))
        psQ = pB.enter_context(tc.tile_pool(name="psQ", bufs=2, space="PSUM"))

        nc.gpsimd.dma_start(out=arh_in[:], in_=hmax_r[:])
        pre_th = []
        for jb in range(8):  # prefetch first 8 h tiles of chunk 0
            th = hb.tile([128, PB], F16, tag="hb", name=f"thp{jb}")
            heng = nc.sync if jb % 2 == 0 else nc.scalar
            heng.dma_start(
                out=th[:], in_=h_dram[jb * 128 : (jb + 1) * 128, 0:PB]
            )
            pre_th.append(th)
        b2_b = b2p.tile([128, c], F32)
        nc.sync.dma_start(
            out=b2_b[:],
            in_=b2_in.ap().rearrange("(o a) -> o a", o=1).to_broadcast((128, c)),
        )
        nc.gpsimd.collective_compute(
            "AllReduce", ALU.max, replica_groups=groups,
            ins=[arh_in.opt()], outs=[arh_out.opt()],
        )
        sh, inv_sh = _derive(arh_out, "h")
        shw2 = scal.tile([128, 1], F32)
        nc.vector.tensor_tensor(out=shw2[:], in0=sh[:], in1=sw2[:],
                                op=ALU.mult)

        # ---------------- phase B ----------------
        n_ob = c // 512  # 2
        for mc in range(n_chb):
            hqT = hqtp.tile([128, ht * PB], BF16, tag="hqT")
            for jb in range(ht):
                if mc == 0 and jb < 8:
                    th = pre_th[jb]
                else:
                    th = hb.tile([128, PB], F16, tag="hb")
                    heng = nc.sync if jb % 2 == 0 else nc.scalar
                    heng.dma_start(
                        out=th[:],
                        in_=h_dram[jb * 128 : (jb + 1) * 128,
                                   mc * PB : (mc + 1) * PB],
                    )
                psq = psQ.tile([128, PB], F32, tag="psQ")
                nc.scalar.activation(
                    out=psq[:], in_=th[:], func=AF.Identity, bias=magic_b[:],
                    scale=inv_sh[:],
                )
                nc.vector.tensor_scalar_add(
                    out=hqT[:, jb * PB : (jb + 1) * PB], in0=psq[:],
                    scalar1=-MAGIC,
                )
            for ms in range(PB // 128):
                pys = [
                    psY.tile([128, 512], F32, tag="psY",
                             name=f"psY{mc}_{ms}_{i}")
                    for i in range(n_ob)
                ]
                prev = None
                for jb in range(ht):
                    for ob in range(n_ob):
                        mmi = nc.tensor.matmul(
                            pys[ob][:],
                            lhsT=hqT[:, jb * PB + ms * 128 : jb * PB + (ms + 1) * 128],
                            rhs=w2qT[:, jb * c + ob * 512 : jb * c + (ob + 1) * 512],
                            start=(jb == 0),
                            stop=(jb == ht - 1),
                        )
                        if prev is not None:
                            _add_dep(mmi.ins, prev.ins, sync=False,
                                     reason="ldw-order")
                        prev = mmi
                for ob in range(n_ob):
                    yt = ys.tile([128, 512], F32, tag="ys")
                    nc.vector.scalar_tensor_tensor(
                        out=yt[:], in0=pys[ob][:], scalar=shw2[:],
                        in1=b2_b[:, ob * 512 : (ob + 1) * 512],
                        op0=ALU.mult, op1=ALU.add,
                    )
                    m0 = mc * PB + ms * 128
                    nc.sync.dma_start(
                        out=y_out[m0 : m0 + 128, ob * 512 : (ob + 1) * 512],
                        in_=yt[:],
                    )
        pB.close()
        late.close()

    if split_waits:
        _split_matmul_waits(nc)
        _dedup_ldweights(nc)
    return nc


_CACHED = {}


def _get_nc(rows, c, h, n_cores, gelu):
    key = (rows, c, h, n_cores, gelu)
    if key not in _CACHED:
        _CACHED[key] = build_nc(rows=rows, c=c, h=h, n_cores=n_cores, gelu=gelu)
    return _CACHED[key]


def run(inputs, trace=False, gelu="Gelu", n_cores=N_CORES):
    x = np.asarray(inputs["x"], np.float32)
    w1 = np.ascontiguousarray(np.asarray(inputs["w1"], np.float32))
    b1 = np.ascontiguousarray(np.asarray(inputs["b1"], np.float32))
    w2 = np.ascontiguousarray(np.asarray(inputs["w2"], np.float32))
    b2 = np.ascontiguousarray(np.asarray(inputs["b2"], np.float32))
    b_, s_, c_ = x.shape
    h_ = w1.shape[0]
    x2d = np.ascontiguousarray(x.reshape(-1, c_))
    rows = x2d.shape[0] // n_cores
    nc = _get_nc(rows, c_, h_, n_cores, gelu)
    in_maps = [
        {
            "x": np.ascontiguousarray(x2d[i * rows : (i + 1) * rows]),
            "w1": w1,
            "b1": b1,
            "w2": w2,
            "b2": b2,
        }
        for i in range(n_cores)
    ]
    res = run_bass_kernel_spmd(nc, in_maps, list(range(n_cores)), trace=trace)
    y2d = np.concatenate([r["y"] for r in res.results], axis=0)
    return y2d.reshape(b_, s_, c_).astype(np.float32), res


def kernel(x, w1, b1, w2, b2):
    y, _ = run({"x": x, "w1": w1, "b1": b1, "w2": w2, "b2": b2})
    return y
